# revision 11
# baseline (speedup 1.0000x reference)
"""Trainium2 Bass kernel: 3x3 stride-1 pad-1 Conv2D, NCHW.

Problem: x (32,128,56,56) f32, weight (256,128,3,3) OIHW, bias (256,)
-> out (32,256,56,56) f32.

Strategy: data-parallel over batch N across 8 NeuronCores (4 images per
core), weights/bias replicated. Per core: implicit GEMM — C_in=128 is
exactly the SBUF partition dim; for each of the 9 filter taps we issue a
128x128 (ci x co-chunk) matmul against a shifted window of the
host-padded image, accumulating all 9 taps into one PSUM bank. fp16
matmul (1 cycle/row) gives ~4x over plain fp32 at ~2.7e-4 rel err.
"""

import numpy as np

import concourse.bass as bass
import concourse.mybir as mybir
import concourse.tile as tile
from concourse import bacc
from concourse.bass_utils import run_bass_kernel_spmd

N_CORES = 8
N_FULL = 32
N_PER_CORE = N_FULL // N_CORES  # 4
CIN = 128
COUT = 256
H = W = 56
HP = WP = 58  # padded spatial
R = 8  # output rows per matmul tile
NT = H // R  # 7 row-tiles per image
NFREE = R * W  # 448 (<= 512 PSUM-bank limit per matmul)
F32 = mybir.dt.float32
F32R = mybir.dt.float32r
F16 = mybir.dt.float16

# Module-level knobs for the dev harness (test.py). The grading harness
# just calls kernel(**inputs) and gets the default (no-trace) path.
TRACE = False
LAST_RESULT = None

_prog = None


def _build_program():
    nc = bacc.Bacc("TRN2", target_bir_lowering=False, debug=False)
    x_d = nc.declare_dram_parameter("x", [N_PER_CORE, CIN, HP * WP], F16, isOutput=False)
    w_d = nc.declare_dram_parameter("wt", [CIN, 9 * COUT], F16, isOutput=False)
    b_d = nc.declare_dram_parameter("bias", [COUT], F32, isOutput=False)
    # fp16 output (|y| <~ 2, rel err 2^-11): halves store traffic + teardown
    # fence latency vs f32; host upcasts.
    out_d = nc.declare_dram_parameter(
        "out", [N_PER_CORE, 2, 128, H * W], F16, isOutput=True
    )

    CH = (R + 2) * WP  # one chunk: R output rows + 2 halo rows of padded input

    with tile.TileContext(nc) as tc:
        with (
            tc.tile_pool(name="const", bufs=1) as const_pool,
            tc.tile_pool(name="xin", bufs=4) as x_pool,
            tc.tile_pool(name="outp", bufs=4) as out_pool,
            tc.tile_pool(name="psum", bufs=7, space="PSUM") as psum_pool,
        ):
            # Weights on the sync engine, c=0 half first (the only half the
            # first 9 matmuls need); x chunks dispatch in parallel on the
            # scalar engine (the second HWDGE-capable engine).
            w_sbs = []
            for c in range(2):
                w_c = const_pool.tile([CIN, 9 * 128], F16, tag=f"w{c}")
                nc.sync.dma_start(
                    out=w_c[:], in_=w_d[:, c * 9 * 128 : (c + 1) * 9 * 128]
                )
                w_sbs.append(w_c)
            bias_sb = const_pool.tile([128, 2], F32)

            # Warmup: dummy matmuls fill the PE during the initial DMA wait, so
            # HAM un-throttles (needs ~3.4us of sustained PE activity) before
            # the first real matmul. Memset on gpsimd (idle at startup) so the
            # busy vector/sync/scalar engines don't gate the warm matmuls.
            scratch = const_pool.tile([128, NFREE], F16)
            nc.gpsimd.memset(scratch[:], 0.0)
            warm_ps = psum_pool.tile([128, NFREE], F32, tag="warm", bufs=1)
            NWARM = 5
            for wi in range(NWARM):
                nc.tensor.matmul(
                    warm_ps[:], lhsT=scratch[:, :128], rhs=scratch[:],
                    start=(wi == 0), stop=(wi == NWARM - 1), skip_group_check=True,
                )

            # Per-image, per-row-block input chunks (overlapping halo rows) so
            # the first matmuls only wait on a ~300KB DMA, not whole images.
            x_view = x_d[:].rearrange("n p (h w) -> n p h w", w=WP)
            x_tiles = {}

            def load_chunk(i, r):
                x_c = x_pool.tile([CIN, CH], F16)
                nc.scalar.dma_start(
                    out=x_c[:],
                    in_=x_view[i][:, r * R : r * R + R + 2, :],
                )
                x_tiles[(i, r)] = x_c

            def compute_tile(i, c, r, row0=0, nrows=R, store_eng=None):
                x_img = x_tiles[(i, r)][:].rearrange("p (h w) -> p h w", w=WP)
                nf = nrows * W
                psum_t = psum_pool.tile([128, NFREE], F32)
                psum_v = psum_t[:, :nf].rearrange("p (r w) -> p r w", w=W)
                for k in range(9):
                    kh, kw = divmod(k, 3)
                    rhs = x_img[:, row0 + kh : row0 + kh + nrows, kw : kw + W]
                    lhsT = w_sbs[c][:, k * 128 : (k + 1) * 128]
                    nc.tensor.matmul(
                        psum_v, lhsT=lhsT, rhs=rhs, start=(k == 0), stop=(k == 8)
                    )
                out_t = out_pool.tile([128, NFREE], F16)
                nc.vector.tensor_scalar_add(
                    out_t[:, :nf], psum_t[:, :nf], bias_sb[:, c : c + 1]
                )
                lo = r * NFREE + row0 * W
                # Alternate store queue: spreads posted writes across both
                # HWDGE queues so neither backs up at the end.
                eng = store_eng or (nc.sync if c == 0 else nc.scalar)
                eng.dma_start(
                    out=out_d[i, c][:, lo : lo + nf], in_=out_t[:, :nf]
                )

            # Emission order = DMA queue order: first two chunks land before
            # compute starts; each chunk is consumed by both co-chunks, then
            # its pool slot recycles.
            load_chunk(0, 0)
            load_chunk(0, 1)
            # Bias is tiny but DMAs as 256 4-byte descriptors; emit it after
            # the critical-path loads (first needed at the first copy-out).
            for c in range(2):
                nc.scalar.dma_start(
                    out=bias_sb[:, c : c + 1],
                    in_=b_d[c * 128 : (c + 1) * 128].rearrange("(p one) -> p one", one=1),
                )
            for i in range(N_PER_CORE):
                for r in range(NT):
                    nxt = (i, r + 2) if r + 2 < NT else (i + 1, (r + 2) % NT)
                    if nxt[0] < N_PER_CORE and nxt not in x_tiles:
                        load_chunk(*nxt)
                    last = i == N_PER_CORE - 1 and r == NT - 1
                    compute_tile(i, 0, r)
                    if last:
                        # Shorten the tail: the final copy-out + store chain
                        # handles 4 rows instead of 8.
                        compute_tile(i, 1, r, row0=0, nrows=4)
                        compute_tile(i, 1, r, row0=4, nrows=4)
                    else:
                        compute_tile(i, 1, r)
                    del x_tiles[(i, r)]
    nc.compile()
    return nc


def kernel(x: np.ndarray, weight: np.ndarray, bias: np.ndarray) -> np.ndarray:
    global _prog, LAST_RESULT
    x = np.ascontiguousarray(x, dtype=np.float32)
    weight = np.ascontiguousarray(weight, dtype=np.float32)
    bias = np.ascontiguousarray(bias, dtype=np.float32)

    # Host-side prep: pad spatial dims, shard batch, pre-transpose weights.
    x_pad = np.zeros((N_FULL, CIN, HP, WP), dtype=np.float16)
    x_pad[:, :, 1:-1, 1:-1] = x
    x_pad = x_pad.reshape(N_FULL, CIN, HP * WP)

    # wt[ci, (c*9 + k)*128 + co2] = weight[c*128 + co2, ci, kh, kw], k = kh*3+kw
    # (c-major so the c=0 half is one contiguous DMA)
    wt = np.ascontiguousarray(
        weight.reshape(2, 128, CIN, 9).transpose(2, 0, 3, 1).reshape(CIN, 9 * COUT)
    ).astype(np.float16)

    if _prog is None:
        _prog = _build_program()

    in_maps = [
        {
            "x": np.ascontiguousarray(x_pad[i * N_PER_CORE : (i + 1) * N_PER_CORE]),
            "wt": wt,
            "bias": bias,
        }
        for i in range(N_CORES)
    ]
    res = run_bass_kernel_spmd(_prog, in_maps, list(range(N_CORES)), trace=TRACE)
    LAST_RESULT = res
    out = np.concatenate([r["out"] for r in res.results], axis=0)
    return out.astype(np.float32).reshape(N_FULL, COUT, H, W)

